# revision 9
# baseline (speedup 1.0000x reference)
"""AdditiveAttention pooling kernel for 8 trn2 NeuronCores.

out = softmax(v @ tanh(h @ W.T + b).T) @ h   for h [131072, 768].

Strategy: shard rows of h across 8 cores. Each core makes a single pass over
its shard computing unnormalized weights a_i = exp(s_i - 10) and the partial
weighted sum P_c = sum_i a_i * h_i. The softmax normalization happens on the
host: out = sum_c P_c / sum_c sum_i a_i  (exact — scores are bounded, so no
running max is required).

All heavy matmuls run in bf16 on the PE; tanh/exp on ACT (one table set);
accumulation in f32 PSUM.
"""

import numpy as np
import ml_dtypes

import concourse.bass as bass
import concourse.tile as tile
from concourse import mybir
from concourse.bass_utils import run_bass_kernel_spmd
from concourse.vector_clock import ScopedClock, VectorClock

N = 131072
H = 768
NCORES = 8
SHARD = N // NCORES          # 16384 rows per core
R = 512                      # rows per block
NB = SHARD // R              # 32 blocks
MC = H // 128                # 6 chunks of the feature dim
QS = R // 128                # 4 row sub-tiles per block
EXP_SHIFT = -10.0            # exp(s - 10): keeps a_i comfortably in f32/bf16

BF16 = mybir.dt.bfloat16
F32 = mybir.dt.float32
NPBF16 = ml_dtypes.bfloat16

_ENGINE_CACHE = {}


def _patch_tail_drain():
    """This walrus build rejects instructions carrying >1 semaphore wait.
    Tile's end-of-context drain aggregates one wait per live processor onto a
    single SP Drain. Split them: one SP nop per pending processor tick (each
    carries exactly one wait), then a wait-free drain. Same engine + program
    order makes this equivalent."""
    if getattr(tile.TileContext, "_ant_tail_patch", False):
        return

    def _drain_and_barrier(self, tick_clock, wait_clock):
        gvc = tick_clock.global_clock
        n = len(gvc)
        for p in range(n):
            t = gvc[p]
            if t > 0:
                req = [0] * n
                req[p] = t
                nop = self.nc.sync.nop()
                wait_clock.add_sem_waits(
                    nop.ins, ScopedClock({None: VectorClock(req)})
                )
        self.nc.sync.drain()
        self.nc.all_engine_barrier()
        popped = self.nc._tile_sem_poison_stack.pop()
        assert popped is self._sem_poison
        self.nc.clear_and_free_semaphores(list(self.sems.allocated().values()))
        self.nc.all_engine_barrier()

    tile.TileContext._drain_and_barrier = _drain_and_barrier

    # The same 1-wait limit applies to every instruction class. Tile's wait
    # assignment can attach 2-3 waits to one instruction (e.g. ACT waiting on
    # a PE tick plus a DMA lane). Before lowering, hoist all but one wait of
    # each instruction onto same-engine NOPs inserted right before it —
    # engine queues are FIFO, so this is an equivalent wait chain.
    orig_lower = tile.TileContext._lower_ordered_insts

    def _lower_with_wait_split(self, ordered):
        for insts in ordered.values():
            out = []
            for inst in insts:
                si = getattr(inst, "sync_info", None)
                if si is not None and len(si.on_wait) > 1:
                    waits = list(si.on_wait)
                    eng = inst.engine
                    for w in waits[:-1]:
                        nop = mybir.InstNoOp(
                            name=self.nc.get_next_instruction_name(),
                            ins=[],
                            outs=[],
                            engine=eng,
                        )
                        nop.sync_info = mybir.SyncInfo(on_wait=[w], on_update=[])
                        out.append(nop)
                    inst.sync_info = mybir.SyncInfo(
                        on_wait=[waits[-1]], on_update=list(si.on_update)
                    )
                out.append(inst)
            insts[:] = out
        return orig_lower(self, ordered)

    tile.TileContext._lower_ordered_insts = _lower_with_wait_split
    tile.TileContext._ant_tail_patch = True


def build_kernel():
    _patch_tail_drain()
    nc = bass.Bass("TRN2", debug=False)

    hT = nc.dram_tensor("hT16", [H, SHARD], BF16, kind="ExternalInput").ap()
    h16 = nc.dram_tensor("h16", [SHARD, H], BF16, kind="ExternalInput").ap()
    wT = nc.dram_tensor("WT16", [H, H], BF16, kind="ExternalInput").ap()
    bT = nc.dram_tensor("bT", [128, MC], F32, kind="ExternalInput").ap()
    vT = nc.dram_tensor("vT", [128, MC], BF16, kind="ExternalInput").ap()
    ctx_out = nc.dram_tensor("ctx_part", [1, H], F32, kind="ExternalOutput").ap()
    alpha_out = nc.dram_tensor(
        "alpha", [128, QS * NB], BF16, kind="ExternalOutput"
    ).ap()

    with tile.TileContext(nc) as tc:
        with tc.tile_pool(name="consts", bufs=1) as consts, \
             tc.tile_pool(name="hTp", bufs=3) as hT_pool, \
             tc.tile_pool(name="h16p", bufs=3) as h16_pool, \
             tc.tile_pool(name="uTp", bufs=3) as uT_pool, \
             tc.tile_pool(name="arow", bufs=2) as arow_pool, \
             tc.tile_pool(name="zps", bufs=2, space="PSUM") as z_pool, \
             tc.tile_pool(name="sps", bufs=2, space="PSUM") as s_pool, \
             tc.tile_pool(name="aps", bufs=2, space="PSUM") as a_pool, \
             tc.tile_pool(name="accps", bufs=1, space="PSUM") as acc_pool, \
             tc.tile_pool(name="outp", bufs=1) as out_pool:

            # --- constants loaded once ---
            wT_sb = consts.tile([128, MC * H], BF16)
            nc.gpsimd.dma_start(
                wT_sb[:].rearrange("p (c j) -> p c j", c=MC),
                wT.rearrange("(c p) j -> p c j", p=128),
            )
            bT_sb = consts.tile([128, MC], F32)
            nc.gpsimd.dma_start(bT_sb[:], bT)
            vT_sb = consts.tile([128, MC], BF16)
            nc.gpsimd.dma_start(vT_sb[:], vT)
            one_sb = consts.tile([1, 1], BF16)
            nc.vector.memset(one_sb[:], 1.0)
            shift_sb = consts.tile([1, 1], F32)
            nc.vector.memset(shift_sb[:], EXP_SHIFT)

            alpha_sb = out_pool.tile([128, QS * NB], BF16)
            # one PSUM bank each; accumulated across all blocks
            acc0 = acc_pool.tile([1, 384], F32, tag="acc0")
            acc1 = acc_pool.tile([1, 384], F32, tag="acc1")
            accs = [acc0, acc1]

            for blk in range(NB):
                i0 = blk * R
                hT_t = hT_pool.tile([128, MC * R], BF16)
                nc.gpsimd.dma_start(
                    hT_t[:].rearrange("p (c i) -> p c i", c=MC),
                    hT[:, i0 : i0 + R].rearrange("(c p) i -> p c i", p=128),
                )
                h16_t = h16_pool.tile([128, QS * H], BF16)
                nc.gpsimd.dma_start(
                    h16_t[:].rearrange("p (q m) -> p q m", q=QS),
                    h16[i0 : i0 + R, :].rearrange("(q p) m -> p q m", p=128),
                )

                s_row = s_pool.tile([1, R], F32)
                for jc in range(MC):
                    z_ps = z_pool.tile([128, R], F32)
                    for mc in range(MC):
                        nc.tensor.matmul(
                            z_ps[:],
                            lhsT=wT_sb[:, mc * H + jc * 128 : mc * H + (jc + 1) * 128],
                            rhs=hT_t[:, mc * R : (mc + 1) * R],
                            start=(mc == 0),
                            stop=(mc == MC - 1),
                        )
                    uT_t = uT_pool.tile([128, R], BF16)
                    nc.scalar.activation(
                        uT_t[:], z_ps[:], mybir.ActivationFunctionType.Tanh,
                        bias=bT_sb[:, jc : jc + 1], scale=1.0,
                    )
                    nc.tensor.matmul(
                        s_row[:],
                        lhsT=vT_sb[:, jc : jc + 1],
                        rhs=uT_t[:],
                        start=(jc == 0),
                        stop=(jc == MC - 1),
                        skip_group_check=True,
                    )

                a_row = arow_pool.tile([1, R], BF16)
                nc.scalar.activation(
                    a_row[:], s_row[:], mybir.ActivationFunctionType.Exp,
                    bias=shift_sb[:], scale=1.0,
                )

                # transpose a_row [1, R] -> alpha column chunks [128, QS]
                a_ps = a_pool.tile([128, QS], F32)
                for q in range(QS):
                    nc.tensor.matmul(
                        a_ps[:, q : q + 1],
                        lhsT=a_row[:, q * 128 : (q + 1) * 128],
                        rhs=one_sb[:],
                        start=True,
                        stop=True,
                        skip_group_check=True,
                    )
                al = alpha_sb[:, blk * QS : (blk + 1) * QS]
                nc.vector.tensor_copy(al, a_ps[:])

                for q in range(QS):
                    for hh in range(2):
                        nc.tensor.matmul(
                            accs[hh][:],
                            lhsT=al[:, q : q + 1],
                            rhs=h16_t[:, q * H + hh * 384 : q * H + (hh + 1) * 384],
                            start=(blk == 0 and q == 0),
                            stop=(blk == NB - 1 and q == QS - 1),
                            skip_group_check=True,
                        )

            ctx_sb = out_pool.tile([1, H], F32)
            nc.vector.tensor_copy(ctx_sb[:, 0:384], acc0[:])
            nc.vector.tensor_copy(ctx_sb[:, 384:768], acc1[:])
            nc.gpsimd.dma_start(ctx_out, ctx_sb[:])
            nc.gpsimd.dma_start(alpha_out, alpha_sb[:])

    return nc


def _get_engine():
    if "nc" not in _ENGINE_CACHE:
        _ENGINE_CACHE["nc"] = build_kernel()
    return _ENGINE_CACHE["nc"]


def make_in_maps(inputs):
    h_i = np.asarray(inputs["h_i"])
    W_weight = np.asarray(inputs["W_weight"])
    W_bias = np.asarray(inputs["W_bias"])
    v = np.asarray(inputs["v"])

    h16 = h_i.astype(NPBF16)
    hT16 = np.ascontiguousarray(h16.T)
    wT16 = np.ascontiguousarray(W_weight.T.astype(NPBF16))
    bT = np.ascontiguousarray(W_bias.astype(np.float32).reshape(MC, 128).T)
    vT = np.ascontiguousarray(v.reshape(MC, 128).T.astype(NPBF16))

    in_maps = []
    for c in range(NCORES):
        r0, r1 = c * SHARD, (c + 1) * SHARD
        in_maps.append({
            "hT16": np.ascontiguousarray(hT16[:, r0:r1]),
            "h16": np.ascontiguousarray(h16[r0:r1, :]),
            "WT16": wT16,
            "bT": bT,
            "vT": vT,
        })
    return in_maps


def kernel(h_i, W_weight, W_bias, v, trace=False):
    in_maps = make_in_maps(
        {"h_i": h_i, "W_weight": W_weight, "W_bias": W_bias, "v": v}
    )
    nc = _get_engine()
    res = run_bass_kernel_spmd(
        nc, in_maps, core_ids=list(range(NCORES)), trace=trace
    )
    _ENGINE_CACHE["last_results"] = res

    num = np.zeros(H, dtype=np.float64)
    den = 0.0
    for c in range(NCORES):
        num += res.results[c]["ctx_part"].astype(np.float64).reshape(H)
        den += res.results[c]["alpha"].astype(np.float64).sum()
    out = (num / den).astype(np.float32).reshape(1, H)
    return out


if __name__ == "__main__":
    rng = np.random.default_rng(0)
    h = rng.standard_normal((N, H), dtype=np.float32)
    W = (rng.standard_normal((H, H)) * 0.02).astype(np.float32)
    b = (rng.standard_normal(H) * 0.02).astype(np.float32)
    vv = (rng.standard_normal((1, H)) * 0.1).astype(np.float32)
    out = kernel(h, W, b, vv)
    # quick numpy reference
    u = np.tanh(h.astype(np.float64) @ W.astype(np.float64).T + b)
    s = (vv.astype(np.float64) @ u.T).ravel()
    a = np.exp(s - s.max())
    ref = (a @ h.astype(np.float64)) / a.sum()
    rel = np.linalg.norm(out.ravel() - ref) / np.linalg.norm(ref)
    print("rel err vs fp64 numpy ref:", rel)
